# revision 34
# baseline (speedup 1.0000x reference)
import numpy as np
import ml_dtypes

import concourse.bass as bass
import concourse.mybir as mybir
import concourse.tile as tile
from concourse.masks import make_identity

F32 = mybir.dt.float32
BF16 = mybir.dt.bfloat16
F8M = mybir.dt.float8e5
BF16_NP = ml_dtypes.bfloat16
E5_NP = ml_dtypes.float8_e5m2

B, DIM, H = 2, 1024, 16
N_FULL = 2048
HD = DIM // H
SCALE = HD ** -0.5
NCORES = 8
H_LOC = H // 4
COLS = H_LOC * HD
KT_D = DIM // 128
GROUPS = [list(range(NCORES))]
MASKVAL = -57344.0


def _patch_tile_drain():
    from bass_rust import ScopedClock

    if getattr(tile.TileContext, "_drain_patched", False):
        return

    def _drain_and_barrier(self, tick_clock, wait_clock):
        nc = self.nc
        drain_inst = nc.sync.drain()
        wait_clock.add_sem_waits(
            drain_inst.ins, ScopedClock({None: tick_clock.global_clock})
        )
        si = drain_inst.ins.sync_info
        if si is not None and len(si.on_wait) > 1:
            waits = list(si.on_wait)
            drain_inst.ins.sync_info = mybir.SyncInfo(
                on_wait=waits[:1], on_update=list(si.on_update)
            )
            for w in waits[1:]:
                d = nc.sync.drain()
                dsi = d.ins.sync_info
                upd = list(dsi.on_update) if dsi is not None else []
                d.ins.sync_info = mybir.SyncInfo(on_wait=[w], on_update=upd)

        nc.all_engine_barrier()
        assert self.sems is not None
        popped = nc._tile_sem_poison_stack.pop()
        assert popped is self._sem_poison
        nc.clear_and_free_semaphores(list(self.sems.allocated().values()))
        nc.all_engine_barrier()

    tile.TileContext._drain_and_barrier = _drain_and_barrier
    tile.TileContext._drain_patched = True


def _split_sync_waits(nc, maxw=1):
    n_split = 0
    for f in nc.m.functions:
        for bb in f.blocks:
            new_insts = []
            for ins in bb.instructions:
                si = ins.sync_info
                if si is not None and len(si.on_wait) > maxw:
                    waits = list(si.on_wait)
                    for i, w in enumerate(waits[maxw:]):
                        nop = mybir.InstNoOp(
                            name=f"{ins.name}-w{i}", ins=[], outs=[]
                        )
                        nop.engine = ins.engine
                        nop.sync_info = mybir.SyncInfo(
                            on_wait=[w], on_update=[]
                        )
                        new_insts.append(nop)
                    ins.sync_info = mybir.SyncInfo(
                        on_wait=waits[:maxw], on_update=list(si.on_update)
                    )
                    n_split += 1
                new_insts.append(ins)
            bb.instructions = new_insts
    return n_split


def build_nc(N=N_FULL, split_waits=True):
    _patch_tile_drain()
    assert N % 512 == 0
    NSLICE = N // 4
    MT = N // 128
    HS = min(N, 1024)
    NH = N // HS
    NT = NSLICE // 128
    NCH = N // 512
    HC = HS // 512

    def nsl_of(nh):
        return slice(HS * nh, HS * (nh + 1))

    nc = bass.Bass(trn_type="TRN2", num_devices=NCORES)

    xqT_e = nc.declare_dram_parameter("xqT", [DIM, N], BF16, isOutput=False)
    xkT_e = nc.declare_dram_parameter("xkT", [DIM, N], BF16, isOutput=False)
    xvT_e = nc.declare_dram_parameter("xvT", [DIM, N], BF16, isOutput=False)
    wq_e = nc.declare_dram_parameter("wq", [DIM, COLS], BF16, isOutput=False)
    wk_e = nc.declare_dram_parameter("wk", [DIM, COLS], BF16, isOutput=False)
    wv_e = nc.declare_dram_parameter("wv", [DIM, COLS], BF16, isOutput=False)
    wpp_e = nc.declare_dram_parameter("wp_pad", [2 * DIM, DIM], BF16, isOutput=False)
    maskT_e = nc.declare_dram_parameter("maskT", [N, N], F8M, isOutput=False)
    bpr_e = nc.declare_dram_parameter("bp_rep", [128, DIM], F32, isOutput=False)
    out_e = nc.declare_dram_parameter("out", [NSLICE, DIM], F32, isOutput=True)

    a2a_in = [nc.dram_tensor(f"a2a_in{i}", [NCORES * 128, NSLICE], BF16)
              for i in range(2)]
    a2a_out = [nc.dram_tensor(f"a2a_out{i}", [NCORES * 128, NSLICE], BF16)
               for i in range(2)]

    with tile.TileContext(nc) as tc:
        with (
            tc.tile_pool(name="cpool", bufs=1) as cpool,
            tc.tile_pool(name="xres", bufs=2) as xres,
            tc.tile_pool(name="pupool", bufs=3) as pupool,
            tc.tile_pool(name="yupool", bufs=4) as yupool,
            tc.tile_pool(name="p3pool", bufs=3) as p3pool,
            tc.tile_pool(name="opool", bufs=2) as opool,
            tc.tile_pool(name="ps", bufs=1, space="PSUM") as ps,
        ):
            SB = [f"SB{i}" for i in range(4)]
            VB = [f"VB{i}" for i in range(2)]

            qt_sb = [cpool.tile([128, N], BF16, tag=f"qt{i}", name=f"qt{i}")
                     for i in range(2)]
            kt_sb = [cpool.tile([128, N], BF16, tag=f"kt{i}", name=f"kt{i}")
                     for i in range(2)]
            vt_sb = [cpool.tile([128, H_LOC, 65], BF16, tag=f"vt{t}",
                                name=f"vt{t}")
                     for t in range(MT)]
            xt_sb = [cpool.tile([64, N], BF16, tag=f"xth{g}", name=f"xth{g}")
                     for g in range(H_LOC)]
            ones_sb = cpool.tile([128, 64], F32, tag="ones", name="ones")
            ident_sb = cpool.tile([128, 128], F8M, tag="ident", name="ident")
            r_sbs = [cpool.tile([65, HS], F32, tag=f"rsum{h}", name=f"rsum{h}")
                     for h in range(2)]
            rr_sb = [cpool.tile([64, HS], BF16, tag=f"rr{h}", name=f"rr{h}")
                     for h in range(2)]
            r32b_sb = [cpool.tile([32, 32], BF16, tag=f"r32b{h}",
                                  name=f"r32b{h}") for h in range(2)]
            r32f_sb = [cpool.tile([32, 32], F32, tag=f"r32f{h}",
                                  name=f"r32f{h}") for h in range(2)]
            mask_sb = cpool.tile([128, MT, N], F8M, tag="mask", name="mask")
            bpr_sb = cpool.tile([128, DIM], F32, tag="bpr", name="bpr")
            wq_sb = cpool.tile([128, KT_D, COLS], BF16, tag="wq", name="wq")
            wk_sb = cpool.tile([128, KT_D, COLS], BF16, tag="wk", name="wk")
            wv_sb = cpool.tile([128, KT_D, COLS], BF16, tag="wv", name="wv")


            wq_v = wq_e[:].rearrange("(kt p) c -> p kt c", p=128)
            wk_v = wk_e[:].rearrange("(kt p) c -> p kt c", p=128)
            wv_v = wv_e[:].rearrange("(kt p) c -> p kt c", p=128)
            nc.sync.dma_start(wq_sb[:], wq_v)
            nc.sync.dma_start(wk_sb[:], wk_v)
            nc.sync.dma_start(wv_sb[:], wv_v)
            nc.sync.dma_start(bpr_sb[:], bpr_e[:])
            nc.gpsimd.memset(ones_sb[:], 0.0)
            nc.gpsimd.memset(ones_sb[64:65, :], 1.0)
            make_identity(nc, ident_sb[:])
            with nc.allow_low_precision(reason="16*I exact in e5m2"):
                nc.scalar.mul(ident_sb[:], ident_sb[:], 16.0)
            for t in range(MT):
                nc.gpsimd.memset(vt_sb[t][:, :, 64:65], 1.0)

            for w_sb, x_e, dst in (
                (wq_sb, xqT_e, qt_sb),
                (wk_sb, xkT_e, kt_sb),
            ):
                x_t = xres.tile([128, KT_D, N], BF16, tag="x", name="x")
                for kt in range(KT_D):
                    nc.sync.dma_start(
                        x_t[:, kt, :], x_e[128 * kt:128 * (kt + 1), :])
                for cb in range(2):
                    psums = [ps.tile([128, 512], F32, tag=SB[i], name="p1qk")
                             for i in range(NCH)]
                    for kt in range(KT_D):
                        for nch in range(NCH):
                            nc.tensor.matmul(
                                psums[nch][:],
                                w_sb[:, kt, 128 * cb:128 * (cb + 1)],
                                x_t[:, kt, 512 * nch:512 * (nch + 1)],
                                start=(kt == 0), stop=(kt == KT_D - 1),
                            )
                    for nch in range(NCH):
                        nc.scalar.copy(
                            dst[cb][:, 512 * nch:512 * (nch + 1)],
                            psums[nch][:],
                        )

            xv_t = xres.tile([128, KT_D, N], BF16, tag="x", name="x")
            nc.sync.dma_start(
                xv_t[:], xvT_e[:].rearrange("(kt p) n -> p kt n", p=128))
            for t in range(MT):
                nc.sync.dma_start(
                    mask_sb[:, t, :], maskT_e[128 * t:128 * (t + 1), :]
                )
            for t in range(MT):
                vps = ps.tile([128, COLS], F32, tag=VB[t % 2], name="p1v")
                for kt in range(KT_D):
                    nc.tensor.matmul(
                        vps[:],
                        xv_t[:, kt, 128 * t:128 * (t + 1)],
                        wv_sb[:, kt, :],
                        start=(kt == 0), stop=(kt == KT_D - 1),
                    )
                nc.scalar.copy(
                    vt_sb[t][:, :, 0:HD],
                    vps[:].rearrange("p (h d) -> p h d", h=H_LOC),
                )

            for hp in range(2):
                for nh in range(NH):
                    nsl = nsl_of(nh)
                    vo = [ps.tile([65, HS], F32, tag=VB[h], name="vo")
                          for h in range(2)]
                    for t in range(MT):
                        s_ps = [[ps.tile([128, 512], F32,
                                         tag=SB[2 * h + ch], name="s")
                                 for ch in range(HC)] for h in range(2)]
                        for ch in range(HC):
                            gsl = slice(HS * nh + 512 * ch,
                                        HS * nh + 512 * (ch + 1))
                            for h in range(2):
                                nc.tensor.matmul(
                                    s_ps[h][ch][:],
                                    kt_sb[hp][64 * h:64 * (h + 1),
                                              128 * t:128 * (t + 1)],
                                    qt_sb[hp][64 * h:64 * (h + 1), gsl],
                                    start=True, stop=False,
                                    tile_position=(64 * h, 0),
                                )
                        for ch in range(HC):
                            gsl = slice(HS * nh + 512 * ch,
                                        HS * nh + 512 * (ch + 1))
                            for h in range(2):
                                nc.tensor.matmul(
                                    s_ps[h][ch][:],
                                    ident_sb[:],
                                    mask_sb[:, t, gsl],
                                    start=False, stop=True,
                                )
                        for h in range(2):
                            for ch in range(HC):
                                csl = slice(512 * ch, 512 * (ch + 1))
                                pu = pupool.tile([128, 512], BF16,
                                                 tag=f"pu{h}{ch}", name="pu")
                                nc.scalar.activation(
                                    pu[:], s_ps[h][ch][:],
                                    mybir.ActivationFunctionType.Exp,
                                    scale=float(SCALE),
                                )
                                nc.tensor.matmul(
                                    vo[h][:, csl],
                                    vt_sb[t][:, 2 * hp + h, :],
                                    pu[:],
                                    start=(t == 0), stop=(t == MT - 1),
                                )
                    for h in range(2):
                        yu = yupool.tile([65, HS], BF16, tag="yu", name="yu")
                        with nc.allow_low_precision(reason="softmax y bf16"):
                            nc.vector.tensor_copy(yu[:], vo[h][:])
                        nc.sync.dma_start(r32b_sb[h][:], yu[64:65, :])
                        nc.vector.reciprocal(r32f_sb[h][:], r32b_sb[h][:])
                        nc.sync.dma_start(r_sbs[h][64:65, :], r32f_sb[h][:])
                        rr_ps = ps.tile([64, HS], F32, tag=VB[h], name="rr")
                        for ch in range(HC):
                            csl = slice(512 * ch, 512 * (ch + 1))
                            nc.tensor.matmul(
                                rr_ps[:, csl],
                                ones_sb[64:65, :],
                                r_sbs[h][64:65, csl],
                                start=True, stop=True,
                            )
                        with nc.allow_low_precision(reason="softmax norm bf16"):
                            nc.vector.tensor_copy(rr_sb[h][:], rr_ps[:])
                        nc.vector.tensor_mul(
                            xt_sb[2 * hp + h][:, nsl],
                            yu[0:64, :],
                            rr_sb[h][:],
                        )
                a2a_in_v = a2a_in[hp][:].rearrange("(j g p) n -> j g p n",
                                                   j=NCORES, g=2)
                for jj in range(NCORES):
                    sl = slice(NSLICE * (jj % 4), NSLICE * (jj % 4 + 1))
                    for g in range(2):
                        nc.sync.dma_start(a2a_in_v[jj, g],
                                          xt_sb[2 * hp + g][:, sl])
                nc.gpsimd.collective_compute(
                    "AllToAll",
                    mybir.AluOpType.bypass,
                    replica_groups=GROUPS,
                    ins=[a2a_in[hp][:]],
                    outs=[a2a_out[hp][:]],
                )
            wpp_v = wpp_e[:].rearrange("(ct p) c -> p ct c", p=128)
            for ch in range(2):
                pjc = [ps.tile([128, 512], F32, tag=SB[nt], name="pj")
                       for nt in range(NT)]
                for half in range(2):
                    half_v = a2a_out[half][:].rearrange("(i p) n -> p i n",
                                                        p=128)
                    for i in range(NCORES):
                        ct = 2 * i + half
                        aa_t = p3pool.tile([128, NSLICE], BF16, tag="aa",
                                           name="aa")
                        nc.sync.dma_start(aa_t[:], half_v[:, i, :])
                        wp_t = p3pool.tile([128, 512], BF16, tag="wp",
                                           name="wp")
                        nc.sync.dma_start(
                            wp_t[:],
                            wpp_v[:, ct, 512 * ch:512 * (ch + 1)])
                        for nt in range(NT):
                            nc.tensor.matmul(
                                pjc[nt][:],
                                aa_t[:, 128 * nt:128 * (nt + 1)],
                                wp_t[:],
                                start=(half == 0 and i == 0),
                                stop=(half == 1 and i == NCORES - 1),
                            )
                csl = slice(512 * ch, 512 * (ch + 1))
                for nt in range(NT):
                    o_t = opool.tile([128, 512], F32, tag=f"ot{ch}",
                                     name="ot")
                    nc.vector.tensor_add(o_t[:], pjc[nt][:], bpr_sb[:, csl])
                    nc.sync.dma_start(
                        out_e[128 * nt:128 * (nt + 1), csl], o_t[:])

    if split_waits:
        _split_sync_waits(nc)
    return nc


def make_in_maps(q, k, v, mask, Wq, Wk, Wv, Wp, bp, N=N_FULL):
    bf = lambda a: np.ascontiguousarray(a).astype(BF16_NP)
    e5 = lambda a: np.ascontiguousarray(a).astype(E5_NP)
    bp_rep = np.ascontiguousarray(
        np.broadcast_to(bp.astype(np.float32), (128, DIM))
    )
    maskT_e5 = [e5((mask[b, 0].T.astype(np.float32) - 1.0) * -MASKVAL)
                for b in range(B)]
    in_maps = []
    for c in range(NCORES):
        b, r = divmod(c, 4)
        cs = slice(COLS * r, COLS * (r + 1))
        wp_pad = np.zeros((2 * DIM, DIM), np.float32)
        wp_pad[DIM * b:DIM * (b + 1)] = Wp
        in_maps.append({
            "xqT": bf(q[b].T),
            "xkT": bf(k[b].T),
            "xvT": bf(v[b].T),
            "wq": bf(Wq[:, cs]),
            "wk": bf(Wk[:, cs]),
            "wv": bf(Wv[:, cs]),
            "wp_pad": bf(wp_pad),
            "maskT": maskT_e5[b],
            "bp_rep": bp_rep,
        })
    return in_maps


def assemble_out(results, N=N_FULL):
    NSLICE = N // 4
    out = np.empty((B, N, DIM), np.float32)
    for c in range(NCORES):
        b, r = divmod(c, 4)
        out[b, NSLICE * r:NSLICE * (r + 1), :] = results[c]["out"]
    return out


_NC_CACHE = {}


def _get_nc():
    if "nc" not in _NC_CACHE:
        _NC_CACHE["nc"] = build_nc()
    return _NC_CACHE["nc"]


def kernel(q, k, v, mask, Wq, Wk, Wv, Wp, bp):
    from concourse.bass_utils import run_bass_kernel_spmd

    q, k, v = (np.asarray(a, np.float32) for a in (q, k, v))
    mask = np.asarray(mask)
    Wq, Wk, Wv, Wp, bp = (
        np.asarray(a, np.float32) for a in (Wq, Wk, Wv, Wp, bp)
    )
    nc = _get_nc()
    in_maps = make_in_maps(q, k, v, mask, Wq, Wk, Wv, Wp, bp)
    res = run_bass_kernel_spmd(nc, in_maps, core_ids=list(range(NCORES)))
    return assemble_out(res.results)


# revision 35
# speedup vs baseline: 1.1163x; 1.1163x over previous
import numpy as np
import ml_dtypes

import concourse.bass as bass
import concourse.mybir as mybir
import concourse.tile as tile

F32 = mybir.dt.float32
BF16 = mybir.dt.bfloat16
F8M = mybir.dt.float8e5
BF16_NP = ml_dtypes.bfloat16
E5_NP = ml_dtypes.float8_e5m2

B, DIM, H = 2, 1024, 16
N_FULL = 2048
HD = DIM // H
SCALE = HD ** -0.5
NCORES = 8
H_LOC = H // 4
COLS = H_LOC * HD
KT_D = DIM // 128
GROUPS = [list(range(NCORES))]


def _patch_tile_drain():
    from bass_rust import ScopedClock

    if getattr(tile.TileContext, "_drain_patched", False):
        return

    def _drain_and_barrier(self, tick_clock, wait_clock):
        nc = self.nc
        drain_inst = nc.sync.drain()
        wait_clock.add_sem_waits(
            drain_inst.ins, ScopedClock({None: tick_clock.global_clock})
        )
        si = drain_inst.ins.sync_info
        if si is not None and len(si.on_wait) > 1:
            waits = list(si.on_wait)
            drain_inst.ins.sync_info = mybir.SyncInfo(
                on_wait=waits[:1], on_update=list(si.on_update)
            )
            for w in waits[1:]:
                d = nc.sync.drain()
                dsi = d.ins.sync_info
                upd = list(dsi.on_update) if dsi is not None else []
                d.ins.sync_info = mybir.SyncInfo(on_wait=[w], on_update=upd)

        nc.all_engine_barrier()
        assert self.sems is not None
        popped = nc._tile_sem_poison_stack.pop()
        assert popped is self._sem_poison
        nc.clear_and_free_semaphores(list(self.sems.allocated().values()))
        nc.all_engine_barrier()

    tile.TileContext._drain_and_barrier = _drain_and_barrier
    tile.TileContext._drain_patched = True


def _split_sync_waits(nc, maxw=1):
    n_split = 0
    for f in nc.m.functions:
        for bb in f.blocks:
            new_insts = []
            for ins in bb.instructions:
                si = ins.sync_info
                if si is not None and len(si.on_wait) > maxw:
                    waits = list(si.on_wait)
                    for i, w in enumerate(waits[maxw:]):
                        nop = mybir.InstNoOp(
                            name=f"{ins.name}-w{i}", ins=[], outs=[]
                        )
                        nop.engine = ins.engine
                        nop.sync_info = mybir.SyncInfo(
                            on_wait=[w], on_update=[]
                        )
                        new_insts.append(nop)
                    ins.sync_info = mybir.SyncInfo(
                        on_wait=waits[:maxw], on_update=list(si.on_update)
                    )
                    n_split += 1
                new_insts.append(ins)
            bb.instructions = new_insts
    return n_split


def build_nc(N=N_FULL, split_waits=True):
    _patch_tile_drain()
    assert N % 512 == 0
    NSLICE = N // 4
    MT = N // 128
    HS = min(N, 1024)
    NH = N // HS
    NT = NSLICE // 128
    NCH = N // 512
    HC = HS // 512

    def nsl_of(nh):
        return slice(HS * nh, HS * (nh + 1))

    nc = bass.Bass(trn_type="TRN2", num_devices=NCORES)

    xqT_e = nc.declare_dram_parameter("xqT", [DIM, N], BF16, isOutput=False)
    xkT_e = nc.declare_dram_parameter("xkT", [DIM, N], BF16, isOutput=False)
    xvT_e = nc.declare_dram_parameter("xvT", [DIM, N], BF16, isOutput=False)
    wq_e = nc.declare_dram_parameter("wq", [DIM, COLS], BF16, isOutput=False)
    wk_e = nc.declare_dram_parameter("wk", [DIM, COLS], BF16, isOutput=False)
    wv_e = nc.declare_dram_parameter("wv", [DIM, COLS], BF16, isOutput=False)
    wpp_e = nc.declare_dram_parameter("wp_pad", [2 * DIM, DIM], BF16, isOutput=False)
    maskT_e = nc.declare_dram_parameter("maskT", [N, N], BF16, isOutput=False)
    bpr_e = nc.declare_dram_parameter("bp_rep", [128, DIM], F32, isOutput=False)
    out_e = nc.declare_dram_parameter("out", [NSLICE, DIM], F32, isOutput=True)

    a2a_in = [nc.dram_tensor(f"a2a_in{i}", [NCORES * 128, NSLICE], BF16)
              for i in range(2)]
    a2a_out = [nc.dram_tensor(f"a2a_out{i}", [NCORES * 128, NSLICE], BF16)
               for i in range(2)]

    with tile.TileContext(nc) as tc:
        with (
            tc.tile_pool(name="cpool", bufs=1) as cpool,
            tc.tile_pool(name="xres", bufs=1) as xres,
            tc.tile_pool(name="pupool", bufs=2) as pupool,
            tc.tile_pool(name="pmpool", bufs=2) as pmpool,
            tc.tile_pool(name="yupool", bufs=4) as yupool,
            tc.tile_pool(name="p3pool", bufs=3) as p3pool,
            tc.tile_pool(name="opool", bufs=2) as opool,
            tc.tile_pool(name="ps", bufs=1, space="PSUM") as ps,
        ):
            SB = [f"SB{i}" for i in range(4)]
            VB = [f"VB{i}" for i in range(2)]

            qt_sb = [cpool.tile([128, N], BF16, tag=f"qt{i}", name=f"qt{i}")
                     for i in range(2)]
            kt_sb = [cpool.tile([128, N], BF16, tag=f"kt{i}", name=f"kt{i}")
                     for i in range(2)]
            vt_sb = [cpool.tile([128, H_LOC, 65], BF16, tag=f"vt{t}",
                                name=f"vt{t}")
                     for t in range(MT)]
            xt_sb = [cpool.tile([64, N], BF16, tag=f"xth{g}", name=f"xth{g}")
                     for g in range(H_LOC)]
            ones_sb = cpool.tile([128, 64], F32, tag="ones", name="ones")
            r_sbs = [cpool.tile([65, HS], F32, tag=f"rsum{h}", name=f"rsum{h}")
                     for h in range(2)]
            rr_sb = [cpool.tile([64, HS], BF16, tag=f"rr{h}", name=f"rr{h}")
                     for h in range(2)]
            r32b_sb = [cpool.tile([32, 32], BF16, tag=f"r32b{h}",
                                  name=f"r32b{h}") for h in range(2)]
            r32f_sb = [cpool.tile([32, 32], F32, tag=f"r32f{h}",
                                  name=f"r32f{h}") for h in range(2)]
            mask_sb = cpool.tile([128, MT, N], BF16, tag="mask", name="mask")
            bpr_sb = cpool.tile([128, DIM], F32, tag="bpr", name="bpr")
            wq_sb = cpool.tile([128, KT_D, COLS], BF16, tag="wq", name="wq")
            wk_sb = cpool.tile([128, KT_D, COLS], BF16, tag="wk", name="wk")
            wv_sb = cpool.tile([128, KT_D, COLS], BF16, tag="wv", name="wv")


            wq_v = wq_e[:].rearrange("(kt p) c -> p kt c", p=128)
            wk_v = wk_e[:].rearrange("(kt p) c -> p kt c", p=128)
            wv_v = wv_e[:].rearrange("(kt p) c -> p kt c", p=128)
            nc.sync.dma_start(wq_sb[:], wq_v)
            nc.sync.dma_start(wk_sb[:], wk_v)
            nc.sync.dma_start(wv_sb[:], wv_v)
            nc.sync.dma_start(bpr_sb[:], bpr_e[:])
            nc.gpsimd.memset(ones_sb[:], 0.0)
            nc.gpsimd.memset(ones_sb[64:65, :], 1.0)
            for t in range(MT):
                nc.gpsimd.memset(vt_sb[t][:, :, 64:65], 1.0)

            for w_sb, x_e, dst in (
                (wq_sb, xqT_e, qt_sb),
                (wk_sb, xkT_e, kt_sb),
            ):
                x_t = xres.tile([128, KT_D, N], BF16, tag="x", name="x")
                for kt in range(KT_D):
                    nc.sync.dma_start(
                        x_t[:, kt, :], x_e[128 * kt:128 * (kt + 1), :])
                for cb in range(2):
                    psums = [ps.tile([128, 512], F32, tag=SB[i], name="p1qk")
                             for i in range(NCH)]
                    for kt in range(KT_D):
                        for nch in range(NCH):
                            nc.tensor.matmul(
                                psums[nch][:],
                                w_sb[:, kt, 128 * cb:128 * (cb + 1)],
                                x_t[:, kt, 512 * nch:512 * (nch + 1)],
                                start=(kt == 0), stop=(kt == KT_D - 1),
                            )
                    for nch in range(NCH):
                        nc.scalar.copy(
                            dst[cb][:, 512 * nch:512 * (nch + 1)],
                            psums[nch][:],
                        )

            xv_t = xres.tile([128, KT_D, N], BF16, tag="x", name="x")
            nc.sync.dma_start(
                xv_t[:], xvT_e[:].rearrange("(kt p) n -> p kt n", p=128))
            for t in range(MT):
                nc.sync.dma_start(
                    mask_sb[:, t, :], maskT_e[128 * t:128 * (t + 1), :]
                )
            for t in range(MT):
                vps = ps.tile([128, COLS], F32, tag=VB[t % 2], name="p1v")
                for kt in range(KT_D):
                    nc.tensor.matmul(
                        vps[:],
                        xv_t[:, kt, 128 * t:128 * (t + 1)],
                        wv_sb[:, kt, :],
                        start=(kt == 0), stop=(kt == KT_D - 1),
                    )
                nc.scalar.copy(
                    vt_sb[t][:, :, 0:HD],
                    vps[:].rearrange("p (h d) -> p h d", h=H_LOC),
                )

            for hp in range(2):
                for nh in range(NH):
                    nsl = nsl_of(nh)
                    vo = [ps.tile([65, HS], F32, tag=VB[h], name="vo")
                          for h in range(2)]
                    for t in range(MT):
                        s_ps = [[ps.tile([128, 512], F32,
                                         tag=SB[2 * h + ch], name="s")
                                 for ch in range(HC)] for h in range(2)]
                        for ch in range(HC):
                            gsl = slice(HS * nh + 512 * ch,
                                        HS * nh + 512 * (ch + 1))
                            for h in range(2):
                                nc.tensor.matmul(
                                    s_ps[h][ch][:],
                                    kt_sb[hp][64 * h:64 * (h + 1),
                                              128 * t:128 * (t + 1)],
                                    qt_sb[hp][64 * h:64 * (h + 1), gsl],
                                    start=True, stop=True,
                                    tile_position=(64 * h, 0),
                                )
                        for h in range(2):
                            for ch in range(HC):
                                csl = slice(512 * ch, 512 * (ch + 1))
                                gsl = slice(HS * nh + 512 * ch,
                                            HS * nh + 512 * (ch + 1))
                                pu = pupool.tile([128, 512], BF16,
                                                 tag=f"pu{h}{ch}", name="pu")
                                nc.scalar.activation(
                                    pu[:], s_ps[h][ch][:],
                                    mybir.ActivationFunctionType.Exp,
                                    scale=float(SCALE),
                                )
                                pm = pmpool.tile([128, 512], BF16,
                                                 tag=f"pm{h}{ch}", name="pm")
                                with nc.allow_low_precision(reason="mask"):
                                    nc.vector.tensor_mul(
                                        pm[:], pu[:], mask_sb[:, t, gsl]
                                    )
                                nc.tensor.matmul(
                                    vo[h][:, csl],
                                    vt_sb[t][:, 2 * hp + h, :],
                                    pm[:],
                                    start=(t == 0), stop=(t == MT - 1),
                                )
                    for h in range(2):
                        yu = yupool.tile([65, HS], BF16, tag="yu", name="yu")
                        with nc.allow_low_precision(reason="softmax y bf16"):
                            nc.vector.tensor_copy(yu[:], vo[h][:])
                        nc.sync.dma_start(r32b_sb[h][:], yu[64:65, :])
                        nc.vector.reciprocal(r32f_sb[h][:], r32b_sb[h][:])
                        nc.sync.dma_start(r_sbs[h][64:65, :], r32f_sb[h][:])
                        rr_ps = ps.tile([64, HS], F32, tag=VB[h], name="rr")
                        for ch in range(HC):
                            csl = slice(512 * ch, 512 * (ch + 1))
                            nc.tensor.matmul(
                                rr_ps[:, csl],
                                ones_sb[64:65, :],
                                r_sbs[h][64:65, csl],
                                start=True, stop=True,
                            )
                        with nc.allow_low_precision(reason="softmax norm bf16"):
                            nc.vector.tensor_copy(rr_sb[h][:], rr_ps[:])
                        nc.vector.tensor_mul(
                            xt_sb[2 * hp + h][:, nsl],
                            yu[0:64, :],
                            rr_sb[h][:],
                        )
                a2a_in_v = a2a_in[hp][:].rearrange("(j g p) n -> j g p n",
                                                   j=NCORES, g=2)
                for jj in range(NCORES):
                    sl = slice(NSLICE * (jj % 4), NSLICE * (jj % 4 + 1))
                    for g in range(2):
                        nc.sync.dma_start(a2a_in_v[jj, g],
                                          xt_sb[2 * hp + g][:, sl])
                nc.gpsimd.collective_compute(
                    "AllToAll",
                    mybir.AluOpType.bypass,
                    replica_groups=GROUPS,
                    ins=[a2a_in[hp][:]],
                    outs=[a2a_out[hp][:]],
                )
            wpp_v = wpp_e[:].rearrange("(ct p) c -> p ct c", p=128)
            for ch in range(2):
                pjc = [ps.tile([128, 512], F32, tag=SB[nt], name="pj")
                       for nt in range(NT)]
                for half in range(2):
                    half_v = a2a_out[half][:].rearrange("(i p) n -> p i n",
                                                        p=128)
                    for i in range(NCORES):
                        ct = 2 * i + half
                        aa_t = p3pool.tile([128, NSLICE], BF16, tag="aa",
                                           name="aa")
                        nc.sync.dma_start(aa_t[:], half_v[:, i, :])
                        wp_t = p3pool.tile([128, 512], BF16, tag="wp",
                                           name="wp")
                        nc.sync.dma_start(
                            wp_t[:],
                            wpp_v[:, ct, 512 * ch:512 * (ch + 1)])
                        for nt in range(NT):
                            nc.tensor.matmul(
                                pjc[nt][:],
                                aa_t[:, 128 * nt:128 * (nt + 1)],
                                wp_t[:],
                                start=(half == 0 and i == 0),
                                stop=(half == 1 and i == NCORES - 1),
                            )
                csl = slice(512 * ch, 512 * (ch + 1))
                for nt in range(NT):
                    o_t = opool.tile([128, 512], F32, tag=f"ot{ch}",
                                     name="ot")
                    nc.vector.tensor_add(o_t[:], pjc[nt][:], bpr_sb[:, csl])
                    nc.sync.dma_start(
                        out_e[128 * nt:128 * (nt + 1), csl], o_t[:])

    if split_waits:
        _split_sync_waits(nc)
    return nc


def make_in_maps(q, k, v, mask, Wq, Wk, Wv, Wp, bp, N=N_FULL):
    bf = lambda a: np.ascontiguousarray(a).astype(BF16_NP)
    e5 = lambda a: np.ascontiguousarray(a).astype(E5_NP)
    bp_rep = np.ascontiguousarray(
        np.broadcast_to(bp.astype(np.float32), (128, DIM))
    )
    maskT_bf = [bf(mask[b, 0].T.astype(np.float32)) for b in range(B)]
    in_maps = []
    for c in range(NCORES):
        b, r = divmod(c, 4)
        cs = slice(COLS * r, COLS * (r + 1))
        wp_pad = np.zeros((2 * DIM, DIM), np.float32)
        wp_pad[DIM * b:DIM * (b + 1)] = Wp
        in_maps.append({
            "xqT": bf(q[b].T),
            "xkT": bf(k[b].T),
            "xvT": bf(v[b].T),
            "wq": bf(Wq[:, cs]),
            "wk": bf(Wk[:, cs]),
            "wv": bf(Wv[:, cs]),
            "wp_pad": bf(wp_pad),
            "maskT": maskT_bf[b],
            "bp_rep": bp_rep,
        })
    return in_maps


def assemble_out(results, N=N_FULL):
    NSLICE = N // 4
    out = np.empty((B, N, DIM), np.float32)
    for c in range(NCORES):
        b, r = divmod(c, 4)
        out[b, NSLICE * r:NSLICE * (r + 1), :] = results[c]["out"]
    return out


_NC_CACHE = {}


def _get_nc():
    if "nc" not in _NC_CACHE:
        _NC_CACHE["nc"] = build_nc()
    return _NC_CACHE["nc"]


def kernel(q, k, v, mask, Wq, Wk, Wv, Wp, bp):
    from concourse.bass_utils import run_bass_kernel_spmd

    q, k, v = (np.asarray(a, np.float32) for a in (q, k, v))
    mask = np.asarray(mask)
    Wq, Wk, Wv, Wp, bp = (
        np.asarray(a, np.float32) for a in (Wq, Wk, Wv, Wp, bp)
    )
    nc = _get_nc()
    in_maps = make_in_maps(q, k, v, mask, Wq, Wk, Wv, Wp, bp)
    res = run_bass_kernel_spmd(nc, in_maps, core_ids=list(range(NCORES)))
    return assemble_out(res.results)


# revision 42
# speedup vs baseline: 1.1268x; 1.0094x over previous
import numpy as np
import ml_dtypes

import concourse.bass as bass
import concourse.mybir as mybir
import concourse.tile as tile

F32 = mybir.dt.float32
BF16 = mybir.dt.bfloat16
BF16_NP = ml_dtypes.bfloat16
E5_NP = ml_dtypes.float8_e5m2

B, DIM, H = 2, 1024, 16
N_FULL = 2048
HD = DIM // H
SCALE = HD ** -0.5
NCORES = 8
H_LOC = H // 4
COLS = H_LOC * HD
KT_D = DIM // 128
GROUPS = [list(range(NCORES))]


def _patch_tile_drain():
    from bass_rust import ScopedClock

    if getattr(tile.TileContext, "_drain_patched", False):
        return

    def _drain_and_barrier(self, tick_clock, wait_clock):
        nc = self.nc
        drain_inst = nc.sync.drain()
        wait_clock.add_sem_waits(
            drain_inst.ins, ScopedClock({None: tick_clock.global_clock})
        )
        si = drain_inst.ins.sync_info
        if si is not None and len(si.on_wait) > 1:
            waits = list(si.on_wait)
            drain_inst.ins.sync_info = mybir.SyncInfo(
                on_wait=waits[:1], on_update=list(si.on_update)
            )
            for w in waits[1:]:
                d = nc.sync.drain()
                dsi = d.ins.sync_info
                upd = list(dsi.on_update) if dsi is not None else []
                d.ins.sync_info = mybir.SyncInfo(on_wait=[w], on_update=upd)

        nc.all_engine_barrier()
        assert self.sems is not None
        popped = nc._tile_sem_poison_stack.pop()
        assert popped is self._sem_poison
        nc.clear_and_free_semaphores(list(self.sems.allocated().values()))
        nc.all_engine_barrier()

    tile.TileContext._drain_and_barrier = _drain_and_barrier
    tile.TileContext._drain_patched = True


def _split_sync_waits(nc, maxw=1):
    n_split = 0
    for f in nc.m.functions:
        for bb in f.blocks:
            new_insts = []
            for ins in bb.instructions:
                si = ins.sync_info
                if si is not None and len(si.on_wait) > maxw:
                    waits = list(si.on_wait)
                    for i, w in enumerate(waits[maxw:]):
                        nop = mybir.InstNoOp(
                            name=f"{ins.name}-w{i}", ins=[], outs=[]
                        )
                        nop.engine = ins.engine
                        nop.sync_info = mybir.SyncInfo(
                            on_wait=[w], on_update=[]
                        )
                        new_insts.append(nop)
                    ins.sync_info = mybir.SyncInfo(
                        on_wait=waits[:maxw], on_update=list(si.on_update)
                    )
                    n_split += 1
                new_insts.append(ins)
            bb.instructions = new_insts
    return n_split


def build_nc(N=N_FULL, split_waits=True):
    _patch_tile_drain()
    assert N % 512 == 0
    NSLICE = N // 4
    MT = N // 128
    HS = min(N, 1024)
    NH = N // HS
    NT = NSLICE // 128
    NCH = N // 512
    HC = HS // 512

    def nsl_of(nh):
        return slice(HS * nh, HS * (nh + 1))

    nc = bass.Bass(trn_type="TRN2", num_devices=NCORES)

    xqT_e = nc.declare_dram_parameter("xqT", [DIM, N], BF16, isOutput=False)
    xkT_e = nc.declare_dram_parameter("xkT", [DIM, N], BF16, isOutput=False)
    xvT_e = nc.declare_dram_parameter("xvT", [DIM, N], BF16, isOutput=False)
    wq_e = nc.declare_dram_parameter("wq", [DIM, COLS], BF16, isOutput=False)
    wk_e = nc.declare_dram_parameter("wk", [DIM, COLS], BF16, isOutput=False)
    wv_e = nc.declare_dram_parameter("wv", [DIM, COLS], BF16, isOutput=False)
    wpp_e = nc.declare_dram_parameter("wp_pad", [2 * DIM, DIM], BF16, isOutput=False)
    maskT_e = nc.declare_dram_parameter("maskT", [N, N], BF16, isOutput=False)
    bpr_e = nc.declare_dram_parameter("bp_rep", [128, DIM], F32, isOutput=False)
    out_e = nc.declare_dram_parameter("out", [NSLICE, DIM], F32, isOutput=True)

    a2a_in = [nc.dram_tensor(f"a2a_in{i}", [NCORES * 128, NSLICE], BF16)
              for i in range(2)]
    a2a_out = [nc.dram_tensor(f"a2a_out{i}", [NCORES * 128, NSLICE], BF16)
               for i in range(2)]

    with tile.TileContext(nc) as tc:
        with (
            tc.tile_pool(name="cpool", bufs=1) as cpool,
            tc.tile_pool(name="xres", bufs=1) as xres,
            tc.tile_pool(name="pupool", bufs=2) as pupool,
            tc.tile_pool(name="pmpool", bufs=2) as pmpool,
            tc.tile_pool(name="yupool", bufs=4) as yupool,
            tc.tile_pool(name="p3pool", bufs=3) as p3pool,
            tc.tile_pool(name="opool", bufs=2) as opool,
            tc.tile_pool(name="ps", bufs=1, space="PSUM") as ps,
        ):
            SB = [f"SB{i}" for i in range(4)]
            VB = [f"VB{i}" for i in range(2)]

            qt_sb = [cpool.tile([128, N], BF16, tag=f"qt{i}", name=f"qt{i}")
                     for i in range(2)]
            kt_sb = [cpool.tile([128, N], BF16, tag=f"kt{i}", name=f"kt{i}")
                     for i in range(2)]
            vt_sb = [cpool.tile([128, H_LOC, 65], BF16, tag=f"vt{t}",
                                name=f"vt{t}")
                     for t in range(MT)]
            xt_sb = [cpool.tile([64, N], BF16, tag=f"xth{g}", name=f"xth{g}")
                     for g in range(H_LOC)]
            ones_sb = cpool.tile([128, 64], F32, tag="ones", name="ones")
            r_sbs = [cpool.tile([65, HS], F32, tag=f"rsum{h}", name=f"rsum{h}")
                     for h in range(2)]
            rr_sb = [cpool.tile([64, HS], BF16, tag=f"rr{h}", name=f"rr{h}")
                     for h in range(2)]
            r32b_sb = [cpool.tile([32, 32], BF16, tag=f"r32b{h}",
                                  name=f"r32b{h}") for h in range(2)]
            r32f_sb = [cpool.tile([32, 32], F32, tag=f"r32f{h}",
                                  name=f"r32f{h}") for h in range(2)]
            mask_sb = cpool.tile([128, MT, N], BF16, tag="mask", name="mask")
            bpr_sb = cpool.tile([128, DIM], F32, tag="bpr", name="bpr")
            wq_sb = cpool.tile([128, KT_D, COLS], BF16, tag="wq", name="wq")
            wk_sb = cpool.tile([128, KT_D, COLS], BF16, tag="wk", name="wk")
            wv_sb = cpool.tile([128, KT_D, COLS], BF16, tag="wv", name="wv")


            wq_v = wq_e[:].rearrange("(kt p) c -> p kt c", p=128)
            wk_v = wk_e[:].rearrange("(kt p) c -> p kt c", p=128)
            wv_v = wv_e[:].rearrange("(kt p) c -> p kt c", p=128)
            nc.sync.dma_start(wq_sb[:], wq_v)
            nc.sync.dma_start(wk_sb[:], wk_v)
            nc.sync.dma_start(wv_sb[:], wv_v)
            nc.sync.dma_start(bpr_sb[:], bpr_e[:])
            nc.gpsimd.memset(ones_sb[:], 0.0)
            nc.gpsimd.memset(ones_sb[64:65, :], 1.0)
            for t in range(MT):
                nc.gpsimd.memset(vt_sb[t][:, :, 64:65], 1.0)

            for w_sb, x_e, dst in (
                (wq_sb, xqT_e, qt_sb),
                (wk_sb, xkT_e, kt_sb),
            ):
                x_t = xres.tile([128, KT_D, N], BF16, tag="x", name="x")
                for kt in range(KT_D):
                    nc.sync.dma_start(
                        x_t[:, kt, :], x_e[128 * kt:128 * (kt + 1), :])
                for cb in range(2):
                    psums = [ps.tile([128, 512], F32, tag=SB[i], name="p1qk")
                             for i in range(NCH)]
                    for kt in range(KT_D):
                        for nch in range(NCH):
                            nc.tensor.matmul(
                                psums[nch][:],
                                w_sb[:, kt, 128 * cb:128 * (cb + 1)],
                                x_t[:, kt, 512 * nch:512 * (nch + 1)],
                                start=(kt == 0), stop=(kt == KT_D - 1),
                            )
                    for nch in range(NCH):
                        nc.scalar.copy(
                            dst[cb][:, 512 * nch:512 * (nch + 1)],
                            psums[nch][:],
                        )

            xv_t = xres.tile([128, KT_D, N], BF16, tag="x", name="x")
            nc.sync.dma_start(
                xv_t[:], xvT_e[:].rearrange("(kt p) n -> p kt n", p=128))
            for t in range(MT):
                nc.sync.dma_start(
                    mask_sb[:, t, :], maskT_e[128 * t:128 * (t + 1), :]
                )
            for t in range(MT):
                vps = ps.tile([128, COLS], F32, tag=VB[t % 2], name="p1v")
                for kt in range(KT_D):
                    nc.tensor.matmul(
                        vps[:],
                        xv_t[:, kt, 128 * t:128 * (t + 1)],
                        wv_sb[:, kt, :],
                        start=(kt == 0), stop=(kt == KT_D - 1),
                    )
                nc.scalar.copy(
                    vt_sb[t][:, :, 0:HD],
                    vps[:].rearrange("p (h d) -> p h d", h=H_LOC),
                )

            a2a_in_v = [a2a_in[i][:].rearrange("(j g p) n -> j g p n",
                                               j=NCORES, g=2)
                        for i in range(2)]
            for hp in range(2):
                for nh in range(NH):
                    nsl = nsl_of(nh)
                    vo = [ps.tile([65, HS], F32, tag=VB[h], name="vo")
                          for h in range(2)]
                    for t in range(MT):
                        s_ps = [[ps.tile([128, 512], F32,
                                         tag=SB[2 * h + ch], name="s")
                                 for ch in range(HC)] for h in range(2)]
                        for ch in range(HC):
                            gsl = slice(HS * nh + 512 * ch,
                                        HS * nh + 512 * (ch + 1))
                            for h in range(2):
                                nc.tensor.matmul(
                                    s_ps[h][ch][:],
                                    kt_sb[hp][64 * h:64 * (h + 1),
                                              128 * t:128 * (t + 1)],
                                    qt_sb[hp][64 * h:64 * (h + 1), gsl],
                                    start=True, stop=True,
                                    tile_position=(64 * h, 0),
                                )
                        for h in range(2):
                            for ch in range(HC):
                                csl = slice(512 * ch, 512 * (ch + 1))
                                gsl = slice(HS * nh + 512 * ch,
                                            HS * nh + 512 * (ch + 1))
                                pu = pupool.tile([128, 512], BF16,
                                                 tag=f"pu{h}{ch}", name="pu")
                                nc.scalar.activation(
                                    pu[:], s_ps[h][ch][:],
                                    mybir.ActivationFunctionType.Exp,
                                    scale=float(SCALE),
                                )
                                pm = pmpool.tile([128, 512], BF16,
                                                 tag=f"pm{h}{ch}", name="pm")
                                with nc.allow_low_precision(reason="mask"):
                                    nc.vector.tensor_mul(
                                        pm[:], pu[:], mask_sb[:, t, gsl]
                                    )
                                nc.tensor.matmul(
                                    vo[h][:, csl],
                                    vt_sb[t][:, 2 * hp + h, :],
                                    pm[:],
                                    start=(t == 0), stop=(t == MT - 1),
                                )
                    for h in range(2):
                        yu = yupool.tile([65, HS], BF16, tag="yu", name="yu")
                        with nc.allow_low_precision(reason="softmax y bf16"):
                            nc.vector.tensor_copy(yu[:], vo[h][:])
                        nc.sync.dma_start(r32b_sb[h][:], yu[64:65, :])
                        nc.vector.reciprocal(r32f_sb[h][:], r32b_sb[h][:])
                        nc.sync.dma_start(r_sbs[h][64:65, :], r32f_sb[h][:])
                        rr_ps = ps.tile([64, HS], F32, tag=VB[h], name="rr")
                        for ch in range(HC):
                            csl = slice(512 * ch, 512 * (ch + 1))
                            nc.tensor.matmul(
                                rr_ps[:, csl],
                                ones_sb[64:65, :],
                                r_sbs[h][64:65, csl],
                                start=True, stop=True,
                            )
                        with nc.allow_low_precision(reason="softmax norm bf16"):
                            nc.vector.tensor_copy(rr_sb[h][:], rr_ps[:])
                        nc.vector.tensor_mul(
                            xt_sb[2 * hp + h][:, nsl],
                            yu[0:64, :],
                            rr_sb[h][:],
                        )
                    for jj in range(NCORES):
                        if (jj % 4) * NSLICE // HS == nh:
                            sl = slice(NSLICE * (jj % 4),
                                       NSLICE * (jj % 4 + 1))
                            for g in range(2):
                                nc.sync.dma_start(a2a_in_v[hp][jj, g],
                                                  xt_sb[2 * hp + g][:, sl])
                nc.gpsimd.collective_compute(
                    "AllToAll",
                    mybir.AluOpType.bypass,
                    replica_groups=GROUPS,
                    ins=[a2a_in[hp][:]],
                    outs=[a2a_out[hp][:]],
                )
            wpp_v = wpp_e[:].rearrange("(ct p) c -> p ct c", p=128)
            for ch in range(2):
                pjc = [ps.tile([128, 512], F32, tag=SB[nt], name="pj")
                       for nt in range(NT)]
                for half in range(2):
                    half_v = a2a_out[half][:].rearrange("(i p) n -> p i n",
                                                        p=128)
                    for i in range(NCORES):
                        ct = 2 * i + half
                        aa_t = p3pool.tile([128, NSLICE], BF16, tag="aa",
                                           name="aa")
                        nc.sync.dma_start(aa_t[:], half_v[:, i, :])
                        wp_t = p3pool.tile([128, 512], BF16, tag="wp",
                                           name="wp")
                        nc.sync.dma_start(
                            wp_t[:],
                            wpp_v[:, ct, 512 * ch:512 * (ch + 1)])
                        for nt in range(NT):
                            nc.tensor.matmul(
                                pjc[nt][:],
                                aa_t[:, 128 * nt:128 * (nt + 1)],
                                wp_t[:],
                                start=(half == 0 and i == 0),
                                stop=(half == 1 and i == NCORES - 1),
                            )
                csl = slice(512 * ch, 512 * (ch + 1))
                for nt in range(NT):
                    o_t = opool.tile([128, 512], F32, tag=f"ot{ch}",
                                     name="ot")
                    nc.vector.tensor_add(o_t[:], pjc[nt][:], bpr_sb[:, csl])
                    nc.sync.dma_start(
                        out_e[128 * nt:128 * (nt + 1), csl], o_t[:])

    if split_waits:
        _split_sync_waits(nc)
    return nc


def make_in_maps(q, k, v, mask, Wq, Wk, Wv, Wp, bp, N=N_FULL):
    bf = lambda a: np.ascontiguousarray(a).astype(BF16_NP)
    e5 = lambda a: np.ascontiguousarray(a).astype(E5_NP)
    bp_rep = np.ascontiguousarray(
        np.broadcast_to(bp.astype(np.float32), (128, DIM))
    )
    maskT_bf = [bf(mask[b, 0].T.astype(np.float32)) for b in range(B)]
    in_maps = []
    for c in range(NCORES):
        b, r = divmod(c, 4)
        cs = slice(COLS * r, COLS * (r + 1))
        wp_pad = np.zeros((2 * DIM, DIM), np.float32)
        wp_pad[DIM * b:DIM * (b + 1)] = Wp
        in_maps.append({
            "xqT": bf(q[b].T),
            "xkT": bf(k[b].T),
            "xvT": bf(v[b].T),
            "wq": bf(Wq[:, cs]),
            "wk": bf(Wk[:, cs]),
            "wv": bf(Wv[:, cs]),
            "wp_pad": bf(wp_pad),
            "maskT": maskT_bf[b],
            "bp_rep": bp_rep,
        })
    return in_maps


def assemble_out(results, N=N_FULL):
    NSLICE = N // 4
    out = np.empty((B, N, DIM), np.float32)
    for c in range(NCORES):
        b, r = divmod(c, 4)
        out[b, NSLICE * r:NSLICE * (r + 1), :] = results[c]["out"]
    return out


_NC_CACHE = {}


def _get_nc():
    if "nc" not in _NC_CACHE:
        _NC_CACHE["nc"] = build_nc()
    return _NC_CACHE["nc"]


def kernel(q, k, v, mask, Wq, Wk, Wv, Wp, bp):
    from concourse.bass_utils import run_bass_kernel_spmd

    q, k, v = (np.asarray(a, np.float32) for a in (q, k, v))
    mask = np.asarray(mask)
    Wq, Wk, Wv, Wp, bp = (
        np.asarray(a, np.float32) for a in (Wq, Wk, Wv, Wp, bp)
    )
    nc = _get_nc()
    in_maps = make_in_maps(q, k, v, mask, Wq, Wk, Wv, Wp, bp)
    res = run_bass_kernel_spmd(nc, in_maps, core_ids=list(range(NCORES)))
    return assemble_out(res.results)


# revision 45
# speedup vs baseline: 1.2834x; 1.1390x over previous
import numpy as np
import ml_dtypes

import concourse.bass as bass
import concourse.mybir as mybir
import concourse.tile as tile

F32 = mybir.dt.float32
BF16 = mybir.dt.bfloat16
BF16_NP = ml_dtypes.bfloat16
E5_NP = ml_dtypes.float8_e5m2

B, DIM, H = 2, 1024, 16
N_FULL = 2048
HD = DIM // H
SCALE = HD ** -0.5
NCORES = 8
H_LOC = H // 4
COLS = H_LOC * HD
KT_D = DIM // 128
GROUPS = [list(range(NCORES))]


def _patch_tile_drain():
    from bass_rust import ScopedClock

    if getattr(tile.TileContext, "_drain_patched", False):
        return

    def _drain_and_barrier(self, tick_clock, wait_clock):
        nc = self.nc
        drain_inst = nc.sync.drain()
        wait_clock.add_sem_waits(
            drain_inst.ins, ScopedClock({None: tick_clock.global_clock})
        )
        si = drain_inst.ins.sync_info
        if si is not None and len(si.on_wait) > 1:
            waits = list(si.on_wait)
            drain_inst.ins.sync_info = mybir.SyncInfo(
                on_wait=waits[:1], on_update=list(si.on_update)
            )
            for w in waits[1:]:
                d = nc.sync.drain()
                dsi = d.ins.sync_info
                upd = list(dsi.on_update) if dsi is not None else []
                d.ins.sync_info = mybir.SyncInfo(on_wait=[w], on_update=upd)

        nc.all_engine_barrier()
        assert self.sems is not None
        popped = nc._tile_sem_poison_stack.pop()
        assert popped is self._sem_poison
        nc.clear_and_free_semaphores(list(self.sems.allocated().values()))
        nc.all_engine_barrier()

    tile.TileContext._drain_and_barrier = _drain_and_barrier
    tile.TileContext._drain_patched = True


def _split_sync_waits(nc, maxw=1):
    n_split = 0
    for f in nc.m.functions:
        for bb in f.blocks:
            new_insts = []
            for ins in bb.instructions:
                si = ins.sync_info
                if si is not None and len(si.on_wait) > maxw:
                    waits = list(si.on_wait)
                    for i, w in enumerate(waits[maxw:]):
                        nop = mybir.InstNoOp(
                            name=f"{ins.name}-w{i}", ins=[], outs=[]
                        )
                        nop.engine = ins.engine
                        nop.sync_info = mybir.SyncInfo(
                            on_wait=[w], on_update=[]
                        )
                        new_insts.append(nop)
                    ins.sync_info = mybir.SyncInfo(
                        on_wait=waits[:maxw], on_update=list(si.on_update)
                    )
                    n_split += 1
                new_insts.append(ins)
            bb.instructions = new_insts
    return n_split


def build_nc(N=N_FULL, split_waits=True):
    _patch_tile_drain()
    assert N % 512 == 0
    NSLICE = N // 4
    MT = N // 128
    HS = min(N, 1024)
    NH = N // HS
    NT = NSLICE // 128
    NCH = N // 512
    HC = HS // 512

    def nsl_of(nh):
        return slice(HS * nh, HS * (nh + 1))

    nc = bass.Bass(trn_type="TRN2", num_devices=NCORES)

    xqT_e = nc.declare_dram_parameter("xqT", [DIM, N], BF16, isOutput=False)
    xkT_e = nc.declare_dram_parameter("xkT", [DIM, N], BF16, isOutput=False)
    xvT_e = nc.declare_dram_parameter("xvT", [DIM, N], BF16, isOutput=False)
    wq_e = nc.declare_dram_parameter("wq", [DIM, COLS], BF16, isOutput=False)
    wk_e = nc.declare_dram_parameter("wk", [DIM, COLS], BF16, isOutput=False)
    wv_e = nc.declare_dram_parameter("wv", [DIM, COLS], BF16, isOutput=False)
    wpp_e = nc.declare_dram_parameter("wp_pad", [2 * DIM, DIM], BF16, isOutput=False)
    maskT_e = nc.declare_dram_parameter("maskT", [N, N], BF16, isOutput=False)
    bpr_e = nc.declare_dram_parameter("bp_rep", [128, DIM], F32, isOutput=False)
    out_e = nc.declare_dram_parameter("out", [NSLICE, DIM], F32, isOutput=True)

    a2a_in = [nc.dram_tensor(f"a2a_in{i}", [NCORES * 128, NSLICE], BF16)
              for i in range(2)]
    a2a_out = [nc.dram_tensor(f"a2a_out{i}", [NCORES * 128, NSLICE], BF16)
               for i in range(2)]

    with tile.TileContext(nc) as tc:
        with (
            tc.tile_pool(name="cpool", bufs=1) as cpool,
            tc.tile_pool(name="xres", bufs=1) as xres,
            tc.tile_pool(name="pupool", bufs=2) as pupool,
            tc.tile_pool(name="pmpool", bufs=2) as pmpool,
            tc.tile_pool(name="yupool", bufs=4) as yupool,
            tc.tile_pool(name="p3pool", bufs=3) as p3pool,
            tc.tile_pool(name="opool", bufs=2) as opool,
            tc.tile_pool(name="ps", bufs=1, space="PSUM") as ps,
        ):
            SA = [f"SA{i}" for i in range(2)]
            VB = [f"VB{i}" for i in range(2)]

            qt_sb = [cpool.tile([128, N], BF16, tag=f"qt{i}", name=f"qt{i}")
                     for i in range(2)]
            kt_sb = [cpool.tile([128, N], BF16, tag=f"kt{i}", name=f"kt{i}")
                     for i in range(2)]
            vt_sb = [cpool.tile([128, H_LOC, 65], BF16, tag=f"vt{t}",
                                name=f"vt{t}")
                     for t in range(MT)]
            xt_sb = [cpool.tile([64, N], BF16, tag=f"xth{g}", name=f"xth{g}")
                     for g in range(H_LOC)]
            ones_sb = cpool.tile([128, 64], F32, tag="ones", name="ones")
            r_sbs = [cpool.tile([65, HS], F32, tag=f"rsum{h}", name=f"rsum{h}")
                     for h in range(2)]
            rr_sb = [cpool.tile([64, HS], BF16, tag=f"rr{h}", name=f"rr{h}")
                     for h in range(2)]
            r32b_sb = [cpool.tile([32, 32], BF16, tag=f"r32b{h}",
                                  name=f"r32b{h}") for h in range(2)]
            r32f_sb = [cpool.tile([32, 32], F32, tag=f"r32f{h}",
                                  name=f"r32f{h}") for h in range(2)]
            mask_sb = cpool.tile([128, MT, N], BF16, tag="mask", name="mask")
            bpr_sb = cpool.tile([128, DIM], F32, tag="bpr", name="bpr")
            wq_sb = cpool.tile([128, KT_D, COLS], BF16, tag="wq", name="wq")
            wk_sb = cpool.tile([128, KT_D, COLS], BF16, tag="wk", name="wk")
            wv_sb = cpool.tile([128, KT_D, COLS], BF16, tag="wv", name="wv")


            wq_v = wq_e[:].rearrange("(kt p) c -> p kt c", p=128)
            wk_v = wk_e[:].rearrange("(kt p) c -> p kt c", p=128)
            wv_v = wv_e[:].rearrange("(kt p) c -> p kt c", p=128)
            nc.sync.dma_start(wq_sb[:], wq_v)
            nc.sync.dma_start(wk_sb[:], wk_v)
            nc.sync.dma_start(wv_sb[:], wv_v)
            nc.sync.dma_start(bpr_sb[:], bpr_e[:])
            nc.gpsimd.memset(ones_sb[:], 0.0)
            nc.gpsimd.memset(ones_sb[64:65, :], 1.0)
            for t in range(MT):
                nc.gpsimd.memset(vt_sb[t][:, :, 64:65], 1.0)

            for w_sb, x_e, dst in (
                (wq_sb, xqT_e, qt_sb),
                (wk_sb, xkT_e, kt_sb),
            ):
                x_t = xres.tile([128, KT_D, N], BF16, tag="x", name="x")
                for kt in range(KT_D):
                    nc.sync.dma_start(
                        x_t[:, kt, :], x_e[128 * kt:128 * (kt + 1), :])
                for cb in range(2):
                    psums = [ps.tile([128, 1024], F32, tag=SA[i],
                                     name="p1qk") for i in range(NCH // 2)]
                    for kt in range(KT_D):
                        for nch in range(NCH):
                            n2, c2 = divmod(nch, 2)
                            nc.tensor.matmul(
                                psums[n2][:, 512 * c2:512 * (c2 + 1)],
                                w_sb[:, kt, 128 * cb:128 * (cb + 1)],
                                x_t[:, kt, 512 * nch:512 * (nch + 1)],
                                start=(kt == 0), stop=(kt == KT_D - 1),
                            )
                    for n2 in range(NCH // 2):
                        nc.scalar.copy(
                            dst[cb][:, 1024 * n2:1024 * (n2 + 1)],
                            psums[n2][:],
                        )

            xv_t = xres.tile([128, KT_D, N], BF16, tag="x", name="x")
            nc.sync.dma_start(
                xv_t[:], xvT_e[:].rearrange("(kt p) n -> p kt n", p=128))
            for t in range(MT):
                nc.sync.dma_start(
                    mask_sb[:, t, :], maskT_e[128 * t:128 * (t + 1), :]
                )
            for t in range(MT):
                vps = ps.tile([128, COLS], F32, tag=VB[t % 2], name="p1v")
                for kt in range(KT_D):
                    nc.tensor.matmul(
                        vps[:],
                        xv_t[:, kt, 128 * t:128 * (t + 1)],
                        wv_sb[:, kt, :],
                        start=(kt == 0), stop=(kt == KT_D - 1),
                    )
                nc.scalar.copy(
                    vt_sb[t][:, :, 0:HD],
                    vps[:].rearrange("p (h d) -> p h d", h=H_LOC),
                )

            for hp in range(2):
                for nh in range(NH):
                    nsl = nsl_of(nh)
                    vo = [ps.tile([65, HS], F32, tag=VB[h], name="vo")
                          for h in range(2)]
                    for t in range(MT):
                        s_ps = [ps.tile([128, HS], F32, tag=SA[h], name="s")
                                for h in range(2)]
                        for ch in range(HC):
                            csl = slice(512 * ch, 512 * (ch + 1))
                            gsl = slice(HS * nh + 512 * ch,
                                        HS * nh + 512 * (ch + 1))
                            for h in range(2):
                                nc.tensor.matmul(
                                    s_ps[h][:, csl],
                                    kt_sb[hp][64 * h:64 * (h + 1),
                                              128 * t:128 * (t + 1)],
                                    qt_sb[hp][64 * h:64 * (h + 1), gsl],
                                    start=True, stop=True,
                                    tile_position=(64 * h, 0),
                                )
                        for h in range(2):
                            pu = pupool.tile([128, HS], BF16,
                                             tag=f"pu{h}", name="pu")
                            nc.scalar.activation(
                                pu[:], s_ps[h][:],
                                mybir.ActivationFunctionType.Exp,
                                scale=float(SCALE),
                            )
                            pm = pmpool.tile([128, HS], BF16,
                                             tag=f"pm{h}", name="pm")
                            with nc.allow_low_precision(reason="mask"):
                                nc.vector.tensor_mul(
                                    pm[:], pu[:], mask_sb[:, t, nsl]
                                )
                            for ch in range(HC):
                                csl = slice(512 * ch, 512 * (ch + 1))
                                nc.tensor.matmul(
                                    vo[h][:, csl],
                                    vt_sb[t][:, 2 * hp + h, :],
                                    pm[:, csl],
                                    start=(t == 0), stop=(t == MT - 1),
                                )
                    for h in range(2):
                        yu = yupool.tile([65, HS], BF16, tag="yu", name="yu")
                        with nc.allow_low_precision(reason="softmax y bf16"):
                            nc.vector.tensor_copy(yu[:], vo[h][:])
                        nc.sync.dma_start(r32b_sb[h][:], yu[64:65, :])
                        nc.vector.reciprocal(r32f_sb[h][:], r32b_sb[h][:])
                        nc.sync.dma_start(r_sbs[h][64:65, :], r32f_sb[h][:])
                        rr_ps = ps.tile([64, HS], F32, tag=VB[h], name="rr")
                        for ch in range(HC):
                            csl = slice(512 * ch, 512 * (ch + 1))
                            nc.tensor.matmul(
                                rr_ps[:, csl],
                                ones_sb[64:65, :],
                                r_sbs[h][64:65, csl],
                                start=True, stop=True,
                            )
                        with nc.allow_low_precision(reason="softmax norm bf16"):
                            nc.vector.tensor_copy(rr_sb[h][:], rr_ps[:])
                        nc.vector.tensor_mul(
                            xt_sb[2 * hp + h][:, nsl],
                            yu[0:64, :],
                            rr_sb[h][:],
                        )
                a2a_in_v = a2a_in[hp][:].rearrange("(j g p) n -> j g p n",
                                                   j=NCORES, g=2)
                for jj in range(NCORES):
                    sl = slice(NSLICE * (jj % 4), NSLICE * (jj % 4 + 1))
                    for g in range(2):
                        nc.sync.dma_start(a2a_in_v[jj, g],
                                          xt_sb[2 * hp + g][:, sl])
                nc.gpsimd.collective_compute(
                    "AllToAll",
                    mybir.AluOpType.bypass,
                    replica_groups=GROUPS,
                    ins=[a2a_in[hp][:]],
                    outs=[a2a_out[hp][:]],
                )
            wpp_v = wpp_e[:].rearrange("(ct p) c -> p ct c", p=128)
            for ch in range(2):
                pjc = [ps.tile([128, 512], F32,
                               tag=[SA[0], SA[1], VB[0], VB[1]][nt],
                               name="pj") for nt in range(NT)]
                for half in range(2):
                    half_v = a2a_out[half][:].rearrange("(i p) n -> p i n",
                                                        p=128)
                    for i in range(NCORES):
                        ct = 2 * i + half
                        aa_t = p3pool.tile([128, NSLICE], BF16, tag="aa",
                                           name="aa")
                        nc.sync.dma_start(aa_t[:], half_v[:, i, :])
                        wp_t = p3pool.tile([128, 512], BF16, tag="wp",
                                           name="wp")
                        nc.sync.dma_start(
                            wp_t[:],
                            wpp_v[:, ct, 512 * ch:512 * (ch + 1)])
                        for nt in range(NT):
                            nc.tensor.matmul(
                                pjc[nt][:],
                                aa_t[:, 128 * nt:128 * (nt + 1)],
                                wp_t[:],
                                start=(half == 0 and i == 0),
                                stop=(half == 1 and i == NCORES - 1),
                            )
                csl = slice(512 * ch, 512 * (ch + 1))
                for nt in range(NT):
                    o_t = opool.tile([128, 512], F32, tag=f"ot{ch}",
                                     name="ot")
                    nc.vector.tensor_add(o_t[:], pjc[nt][:], bpr_sb[:, csl])
                    nc.sync.dma_start(
                        out_e[128 * nt:128 * (nt + 1), csl], o_t[:])

    if split_waits:
        _split_sync_waits(nc)
    return nc


def make_in_maps(q, k, v, mask, Wq, Wk, Wv, Wp, bp, N=N_FULL):
    bf = lambda a: np.ascontiguousarray(a).astype(BF16_NP)
    e5 = lambda a: np.ascontiguousarray(a).astype(E5_NP)
    bp_rep = np.ascontiguousarray(
        np.broadcast_to(bp.astype(np.float32), (128, DIM))
    )
    maskT_bf = [bf(mask[b, 0].T.astype(np.float32)) for b in range(B)]
    in_maps = []
    for c in range(NCORES):
        b, r = divmod(c, 4)
        cs = slice(COLS * r, COLS * (r + 1))
        wp_pad = np.zeros((2 * DIM, DIM), np.float32)
        wp_pad[DIM * b:DIM * (b + 1)] = Wp
        in_maps.append({
            "xqT": bf(q[b].T),
            "xkT": bf(k[b].T),
            "xvT": bf(v[b].T),
            "wq": bf(Wq[:, cs]),
            "wk": bf(Wk[:, cs]),
            "wv": bf(Wv[:, cs]),
            "wp_pad": bf(wp_pad),
            "maskT": maskT_bf[b],
            "bp_rep": bp_rep,
        })
    return in_maps


def assemble_out(results, N=N_FULL):
    NSLICE = N // 4
    out = np.empty((B, N, DIM), np.float32)
    for c in range(NCORES):
        b, r = divmod(c, 4)
        out[b, NSLICE * r:NSLICE * (r + 1), :] = results[c]["out"]
    return out


_NC_CACHE = {}


def _get_nc():
    if "nc" not in _NC_CACHE:
        _NC_CACHE["nc"] = build_nc()
    return _NC_CACHE["nc"]


def kernel(q, k, v, mask, Wq, Wk, Wv, Wp, bp):
    from concourse.bass_utils import run_bass_kernel_spmd

    q, k, v = (np.asarray(a, np.float32) for a in (q, k, v))
    mask = np.asarray(mask)
    Wq, Wk, Wv, Wp, bp = (
        np.asarray(a, np.float32) for a in (Wq, Wk, Wv, Wp, bp)
    )
    nc = _get_nc()
    in_maps = make_in_maps(q, k, v, mask, Wq, Wk, Wv, Wp, bp)
    res = run_bass_kernel_spmd(nc, in_maps, core_ids=list(range(NCORES)))
    return assemble_out(res.results)
